# revision 7
# baseline (speedup 1.0000x reference)
"""Trainium2 Bass kernel for nn_Decoder (RNN decoder w/ Luong attention).

Reference computation (B=64, S=64, H=E=512, V=8000):
    tokens  = [SOS, target[:, 1:]]
    x_seq   = emb[tokens]
    h_0     = encoder_hiddens[:, -1]
    per step t:
        h_t    = tanh(x_t @ W_ih.T + h_{t-1} @ W_hh.T + b_cell)
        o_t    = h_t @ W_ho.T + b_ho
        scores = einsum("bsh,bh->bs", enc, h_t); w = softmax(scores)
        ctx_t  = einsum("bs,bsh->bh", w, enc)
        out_t  = tanh([o_t, ctx_t] @ W_c.T + b_c)

Strategy: fully data-parallel over batch across 8 cores (8 batch rows each),
weights replicated, zero collectives. Per core, with M = 8*64 = 512 local
(t, b) rows:
  phase C: C^T = W_ihT_aug.T @ XT_aug  (input projection for all steps,
           b_cell folded in via an augmented ones row)
  phase 1: sequential recurrence, feature-major h^T (H on partitions,
           batch on free); C added in PSUM via an identity matmul; one
           tanh ACT per step.
  phase 2: o^T = W_ho^T.T @ H  written directly into the phase-3
           stationary buffer XT (no transposes anywhere).
  attn:    scores computed transposed (s on partitions) so the softmax
           denominator comes from a ones-matmul; ctx^T lands in XT.
  phase 3: OUT = XT.T @ W_cT_pad streamed from HBM once (fp16, ~139MB),
           b_c (+ W_c[:, :V] @ b_ho, folded on host) via the ones row,
           tanh on ACT, fp32 out.

All matmul operands fp16 (full PE rate, 10-bit mantissa); softmax exp/ctx
path in fp32.
"""

import os
from contextlib import ExitStack

import numpy as np

# ---- problem constants (hardcoded per harness contract) ----
B, S, H, E, V = 64, 64, 512, 512, 8000
SOS_IDX = 1
NCORES = 8
BL = B // NCORES          # local batch = 8
M = BL * S                # local rows = 512, r = t*BL + b
P = 128                   # partitions

EKT = 5                   # k-tiles for E+bias (640 rows)
HKT = H // P              # 4
# phase-3 contraction layout: [o (8000) | pad (64) | ctx (512) | ones | pad]
KO_PAD = 8064             # o rows padded
K3 = 8704                 # total phase-3 contraction rows (68 tiles)
K3T = K3 // P             # 68
CTX_T0 = KO_PAD // P      # 63: first ctx k-tile
ONES_ROW = KO_PAD + H     # 8576 (tile 67, partition 0)
VT_FULL = V // P          # 62 full 128-row vocab tiles for o^T
V_REM = V - VT_FULL * P   # 64
NW = 500                  # phase-3 vocab chunk width
NCH = V // NW             # 16
SCORE_CLAMP = 80.0

_f16 = None
_f32 = None

_cache = {}


def _build_program(phases="c123"):
    import concourse.bass as bass
    import concourse.tile as tile
    import concourse.mybir as mybir
    from concourse import bacc

    f16 = mybir.dt.float16
    f32 = mybir.dt.float32
    Act = mybir.ActivationFunctionType

    nc = bacc.Bacc("TRN2", target_bir_lowering=False, debug=False,
                   num_devices=NCORES)

    xt = nc.dram_tensor("xt", [EKT * P, M], f16, kind="ExternalInput").ap()
    wih = nc.dram_tensor("wih", [EKT * P, H], f16, kind="ExternalInput").ap()
    whh = nc.dram_tensor("whh", [H, H], f16, kind="ExternalInput").ap()
    h0 = nc.dram_tensor("h0", [H, BL], f16, kind="ExternalInput").ap()
    ent = nc.dram_tensor("ent", [BL, H, S], f16, kind="ExternalInput").ap()
    enn = nc.dram_tensor("enn", [BL, S, H], f32, kind="ExternalInput").ap()
    who = nc.dram_tensor("who", [H, V], f16, kind="ExternalInput").ap()
    wct = nc.dram_tensor("wct", [K3, V], f16, kind="ExternalInput").ap()
    ident = nc.dram_tensor("ident", [P, P], f16, kind="ExternalInput").ap()
    out = nc.dram_tensor("out", [M, V], f32, kind="ExternalOutput").ap()

    with tile.TileContext(nc) as tc, ExitStack() as ctx:
        ts = bass.ts

        # ---------------- persistent SBUF ----------------
        const_pool = ctx.enter_context(tc.tile_pool(name="consts", bufs=1))
        xt_pool = ctx.enter_context(tc.tile_pool(name="xtp", bufs=1))

        ident_sb = const_pool.tile([P, P], f16)
        nc.sync.dma_start(ident_sb[:], ident[:])
        ones_sb = const_pool.tile([S, P], f32)   # softmax-sum broadcaster
        nc.vector.memset(ones_sb[:], 1.0)

        XT = xt_pool.tile([P, K3T, M], f16)      # phase-3 stationary
        H_sb = xt_pool.tile([P, HKT, M], f16)    # h^T for all local rows

        # zero XT pad regions (rest is fully written by phases 2/attn)
        nc.vector.memset(XT[V_REM:, VT_FULL, :], 0.0)
        nc.vector.memset(XT[:, K3T - 1, :], 0.0)
        nc.vector.memset(XT[0:1, K3T - 1, :], 1.0)

        with tc.tile_pool(name="ph12", bufs=1) as p12, \
             tc.tile_pool(name="psA", bufs=2, space="PSUM") as psA:
            xt_sb = p12.tile([P, EKT, M], f16)
            nc.sync.dma_start(xt_sb[:], xt.rearrange("(k p) m -> p k m", p=P))
            wih_sb = p12.tile([P, EKT, H], f16)
            nc.sync.dma_start(wih_sb[:], wih.rearrange("(k p) m -> p k m", p=P))
            whh_sb = p12.tile([P, HKT, H], f16)
            nc.sync.dma_start(whh_sb[:], whh.rearrange("(k p) m -> p k m", p=P))
            h0_sb = p12.tile([P, HKT, BL], f16)
            nc.sync.dma_start(h0_sb[:], h0.rearrange("(k p) b -> p k b", p=P))
            C_sb = p12.tile([P, HKT, M], f16)

            # ---------------- phase C: input projection ----------------
            for m in range(HKT if "c" in phases else 0):
                pc = psA.tile([P, M], f32, tag="pc")
                for k in range(EKT):
                    nc.tensor.matmul(pc[:], wih_sb[:, k, ts(m, P)],
                                     xt_sb[:, k, :],
                                     start=(k == 0), stop=(k == EKT - 1))
                nc.vector.tensor_copy(C_sb[:, m, :], pc[:])

            # ---------------- phase 1: recurrence ----------------
            for t in range(S if "1" in phases else 0):
                ph = psA.tile([P, HKT, BL], f32, tag="ph")
                for m in range(HKT):
                    for k in range(HKT):
                        rhs = (h0_sb[:, k, :] if t == 0
                               else H_sb[:, k, ts(t - 1, BL)])
                        nc.tensor.matmul(ph[:, m, :],
                                         whh_sb[:, k, ts(m, P)], rhs,
                                         start=(k == 0), stop=False)
                    nc.tensor.matmul(ph[:, m, :], ident_sb[:],
                                     C_sb[:, m, ts(t, BL)],
                                     start=False, stop=True)
                nc.scalar.activation(H_sb[:, :, ts(t, BL)], ph[:], Act.Tanh)

        # ---------------- phase 2: o^T into XT ----------------
        with tc.tile_pool(name="ph2", bufs=1) as p2, \
             tc.tile_pool(name="psB", bufs=1, space="PSUM") as psB:
            who_sb = p2.tile([P, HKT, V], f16)
            nc.sync.dma_start(who_sb[:], who.rearrange("(k p) v -> p k v", p=P))

            for v in range(VT_FULL + 1 if "2" in phases else 0):
                w = P if v < VT_FULL else V_REM
                po = psB.tile([P, M], f32, tag="po", bufs=3)
                for k in range(HKT):
                    nc.tensor.matmul(po[:w, :],
                                     who_sb[:, k, bass.ds(v * P, w)],
                                     H_sb[:, k, :],
                                     start=(k == 0), stop=(k == HKT - 1))
                nc.vector.tensor_copy(XT[:w, v, :], po[:w, :])

            # ---------------- attention ----------------
            ent_sb = p2.tile([P, BL, HKT, S], f16)
            nc.sync.dma_start(ent_sb[:],
                              ent.rearrange("b (k p) s -> p b k s", p=P))
            enn_sb = p2.tile([S, BL, H], f32)
            nc.sync.dma_start(enn_sb[:], enn.rearrange("b s h -> s b h"))

            for b in range(BL if "a" in phases else 0):
                hloc = [H_sb[:, kk, :].rearrange("p (t b) -> p t b", b=BL)[:, :, b]
                        for kk in range(HKT)]
                pscr = psB.tile([S, S], f32, tag="pscr", bufs=2)
                for k in range(HKT):
                    nc.tensor.matmul(pscr[:], ent_sb[:, b, k, :], hloc[k],
                                     start=(k == 0), stop=(k == HKT - 1))
                sc_sb = p2.tile([S, S], f32, tag="scs")
                nc.vector.tensor_scalar_min(sc_sb[:], pscr[:], SCORE_CLAMP)
                ex_sb = p2.tile([S, S], f32, tag="exs")
                nc.scalar.activation(ex_sb[:], sc_sb[:], Act.Exp)
                # column sums broadcast to all partitions via ones-matmul
                psum_bc = psB.tile([P, S], f32, tag="pbc", bufs=1)
                nc.tensor.matmul(psum_bc[:], ones_sb[:], ex_sb[:],
                                 start=True, stop=True)
                rbc_sb = p2.tile([P, S], f32, tag="rbc")
                nc.vector.reciprocal(rbc_sb[:], psum_bc[:])
                for j in range(HKT):
                    pctx = psB.tile([P, S], f32, tag="pctx", bufs=2)
                    nc.tensor.matmul(pctx[:], enn_sb[:, b, ts(j, P)], ex_sb[:],
                                     start=True, stop=True)
                    xslice = XT[:, CTX_T0 + j, :].rearrange(
                        "p (t b) -> p t b", b=BL)[:, :, b]
                    nc.vector.tensor_mul(xslice, pctx[:], rbc_sb[:])

        # ---------------- phase 3: big W_c matmul ----------------
        with tc.tile_pool(name="ph3", bufs=4) as p3, \
             tc.tile_pool(name="ph3o", bufs=4) as p3o, \
             tc.tile_pool(name="psC", bufs=8, space="PSUM") as psC:
            for n in range(NCH if "3" in phases else 0):
                pts = [psC.tile([P, NW], f32, tag="p3", name=f"p3_{n}_{m}")
                       for m in range(HKT)]
                for k in range(K3T):
                    rhs = p3.tile([P, NW], f16, tag="rhs")
                    nc.sync.dma_start(rhs[:], wct[ts(k, P), ts(n, NW)])
                    for m in range(HKT):
                        nc.tensor.matmul(pts[m][:], XT[:, k, ts(m, P)], rhs[:],
                                         start=(k == 0), stop=(k == K3T - 1))
                for m in range(HKT):
                    ot = p3o.tile([P, NW], f32, tag="ot")
                    nc.scalar.activation(ot[:], pts[m][:], Act.Tanh)
                    nc.sync.dma_start(out[ts(m, P), ts(n, NW)], ot[:])

    nc.compile()
    return nc


def _host_prep(target, encoder_hiddens, emb, W_ih, W_hh, b_cell, W_ho, b_ho,
               W_c, b_c):
    f16 = np.float16
    tok = np.asarray(target).astype(np.int64).copy()
    tok[:, 0] = SOS_IDX
    x = np.asarray(emb, np.float32)[tok]            # (B, S, E)
    enc = np.asarray(encoder_hiddens, np.float32)

    wih_a = np.zeros((EKT * P, H), f16)
    wih_a[:E] = W_ih.T.astype(f16)                  # (E, H)
    wih_a[E] = np.asarray(b_cell, np.float32).astype(f16)
    whh_a = np.asarray(W_hh, np.float32).T.astype(f16)
    who_a = np.ascontiguousarray(np.asarray(W_ho, np.float32).T).astype(f16)

    W_c = np.asarray(W_c, np.float32)
    b_c_eff = np.asarray(b_c, np.float32) + W_c[:, :V] @ np.asarray(b_ho, np.float32)
    wct_a = np.zeros((K3, V), f16)
    wct_a[:V] = W_c[:, :V].T.astype(f16)
    wct_a[KO_PAD:KO_PAD + H] = W_c[:, V:].T.astype(f16)
    wct_a[ONES_ROW] = b_c_eff.astype(f16)

    ident_a = np.eye(P, dtype=f16)

    shared = {"wih": wih_a, "whh": whh_a, "who": who_a, "wct": wct_a,
              "ident": ident_a}

    in_maps = []
    for c in range(NCORES):
        sl = slice(c * BL, (c + 1) * BL)
        xb = x[sl]                                   # (BL, S, E)
        xt_a = np.zeros((EKT * P, M), f16)
        # col r = t*BL + b
        xt_a[:E] = xb.transpose(2, 1, 0).reshape(E, M).astype(f16)
        xt_a[E] = 1.0
        encb = enc[sl]                               # (BL, S, H)
        in_maps.append(dict(
            shared,
            xt=xt_a,
            h0=np.ascontiguousarray(encb[:, -1].T).astype(f16),
            ent=np.ascontiguousarray(encb.transpose(0, 2, 1)).astype(f16),
            enn=np.ascontiguousarray(encb),
        ))
    return in_maps


def _get_program():
    if "nc" not in _cache:
        _cache["nc"] = _build_program(os.environ.get("KERNEL_PHASES", "c12a3"))
    return _cache["nc"]


def kernel(**inputs):
    from concourse.bass_utils import run_bass_kernel_spmd

    nc = _get_program()
    in_maps = _host_prep(**inputs)
    res = run_bass_kernel_spmd(nc, in_maps, core_ids=list(range(NCORES)))
    _cache["last_result"] = res

    outp = np.empty((B, S, V), np.float32)
    for c in range(NCORES):
        loc = res.results[c]["out"]                  # (M, V), r = t*BL + b
        outp[c * BL:(c + 1) * BL] = loc.reshape(S, BL, V).transpose(1, 0, 2)
    return outp


if __name__ == "__main__":
    rng = np.random.default_rng(0)
    ins = {
        "target": rng.integers(0, V, (B, S)),
        "encoder_hiddens": rng.standard_normal((B, S, H)).astype(np.float32),
        "emb": rng.standard_normal((V, E)).astype(np.float32),
        "W_ih": (rng.standard_normal((H, E)) / np.sqrt(E)).astype(np.float32),
        "W_hh": (rng.standard_normal((H, H)) / np.sqrt(H)).astype(np.float32),
        "b_cell": np.zeros(H, np.float32),
        "W_ho": (rng.standard_normal((V, H)) / np.sqrt(H)).astype(np.float32),
        "b_ho": np.zeros(V, np.float32),
        "W_c": (rng.standard_normal((V, V + H)) / np.sqrt(V + H)).astype(np.float32),
        "b_c": np.zeros(V, np.float32),
    }
    o = kernel(**ins)
    print("kernel ran, output shape", o.shape, "finite:", np.isfinite(o).all())


# revision 12
# speedup vs baseline: 1.0521x; 1.0521x over previous
"""Trainium2 Bass kernel for nn_Decoder (RNN decoder w/ Luong attention).

Reference computation (B=64, S=64, H=E=512, V=8000):
    tokens  = [SOS, target[:, 1:]]
    x_seq   = emb[tokens]
    h_0     = encoder_hiddens[:, -1]
    per step t:
        h_t    = tanh(x_t @ W_ih.T + h_{t-1} @ W_hh.T + b_cell)
        o_t    = h_t @ W_ho.T + b_ho
        scores = einsum("bsh,bh->bs", enc, h_t); w = softmax(scores)
        ctx_t  = einsum("bs,bsh->bh", w, enc)
        out_t  = tanh([o_t, ctx_t] @ W_c.T + b_c)

Strategy: fully data-parallel over batch across 8 cores (8 batch rows each),
weights replicated, zero collectives. Per core, with M = 8*64 = 512 local
(t, b) rows:
  phase C: C^T = W_ihT_aug.T @ XT_aug  (input projection for all steps,
           b_cell folded in via an augmented ones row)
  phase 1: sequential recurrence, feature-major h^T (H on partitions,
           batch on free); C added in PSUM via an identity matmul; one
           tanh ACT per step.
  phase 2: o^T = W_ho^T.T @ H  written directly into the phase-3
           stationary buffer XT (no transposes anywhere).
  attn:    scores computed transposed (s on partitions) so the softmax
           denominator comes from a ones-matmul; ctx^T lands in XT.
  phase 3: OUT = XT.T @ W_cT_pad streamed from HBM once (fp16, ~139MB),
           b_c (+ W_c[:, :V] @ b_ho, folded on host) via the ones row,
           tanh on ACT, fp32 out.

All matmul operands fp16 (full PE rate, 10-bit mantissa); softmax exp/ctx
path in fp32.
"""

import os
from contextlib import ExitStack

import numpy as np

# ---- problem constants (hardcoded per harness contract) ----
B, S, H, E, V = 64, 64, 512, 512, 8000
SOS_IDX = 1
NCORES = 8
BL = B // NCORES          # local batch = 8
M = BL * S                # local rows = 512, r = t*BL + b
P = 128                   # partitions

EKT = 5                   # k-tiles for E+bias (640 rows)
HKT = H // P              # 4
# phase-3 contraction layout: [o (8000) | ctx (512) | ones row | zero pad]
K3 = 8576                 # total phase-3 contraction rows (67 tiles)
K3T = K3 // P             # 67
ONES_ROW = V + H          # 8512 (tile 66, partition 64)
VT_FULL = V // P          # 62 full 128-row vocab tiles for o^T
V_REM = V - VT_FULL * P   # 64
NW = 500                  # phase-3 vocab chunk width
NCH = V // NW             # 16
SCORE_CLAMP = 80.0

_f16 = None
_f32 = None

_cache = {}


def _build_program(phases="c123"):
    import concourse.bass as bass
    import concourse.tile as tile
    import concourse.mybir as mybir
    from concourse import bacc

    f16 = mybir.dt.float16
    f32 = mybir.dt.float32
    Act = mybir.ActivationFunctionType

    nc = bacc.Bacc("TRN2", target_bir_lowering=False, debug=False,
                   num_devices=NCORES)

    xt = nc.dram_tensor("xt", [EKT * P, M], f16, kind="ExternalInput").ap()
    wih = nc.dram_tensor("wih", [EKT * P, H], f16, kind="ExternalInput").ap()
    whh = nc.dram_tensor("whh", [H, H], f16, kind="ExternalInput").ap()
    h0 = nc.dram_tensor("h0", [H, BL], f16, kind="ExternalInput").ap()
    ent = nc.dram_tensor("ent", [BL, H, S], f16, kind="ExternalInput").ap()
    enn = nc.dram_tensor("enn", [BL, S, H], f32, kind="ExternalInput").ap()
    who = nc.dram_tensor("who", [H, V], f16, kind="ExternalInput").ap()
    wct = nc.dram_tensor("wct", [K3T, NCH, P, NW], f16,
                         kind="ExternalInput").ap()
    ident = nc.dram_tensor("ident", [P, P], f16, kind="ExternalInput").ap()
    out = nc.dram_tensor("out", [M, V], f32, kind="ExternalOutput").ap()
    debug = os.environ.get("KERNEL_DEBUG_OUT", "0") == "1"
    if debug:
        hdump = nc.dram_tensor("hdump", [P, HKT, M], f16,
                               kind="ExternalOutput").ap()
        xdump = nc.dram_tensor("xdump", [P, K3T, M], f16,
                               kind="ExternalOutput").ap()

    with tile.TileContext(nc) as tc, ExitStack() as ctx:
        ts = bass.ts

        # ---------------- persistent SBUF ----------------
        const_pool = ctx.enter_context(tc.tile_pool(name="consts", bufs=1))
        xt_pool = ctx.enter_context(tc.tile_pool(name="xtp", bufs=1))

        ident_sb = const_pool.tile([P, P], f16)
        nc.sync.dma_start(ident_sb[:], ident[:])
        ones_sb = const_pool.tile([S, P], f32)   # softmax-sum broadcaster
        nc.vector.memset(ones_sb[:], 1.0)

        XT = xt_pool.tile([P, K3T, M], f16)      # phase-3 stationary
        H_sb = xt_pool.tile([P, HKT, M], f16)    # h^T for all local rows

        # zero XT pad region + ones row (rest fully written by phases 2/attn)
        nc.vector.memset(XT[V_REM:, K3T - 1, :], 0.0)
        nc.vector.memset(XT[V_REM:V_REM + 1, K3T - 1, :], 1.0)

        with tc.tile_pool(name="ph12", bufs=1) as p12, \
             tc.tile_pool(name="psA", bufs=2, space="PSUM") as psA:
            xt_r = xt.rearrange("(k p) m -> p k m", p=P)
            xt_sb = p12.tile([P, EKT, M], f16)
            wih_r = wih.rearrange("(k p) m -> p k m", p=P)
            wih_sb = p12.tile([P, EKT, H], f16)
            for k in range(EKT):
                nc.sync.dma_start(wih_sb[:, k, :], wih_r[:, k, :])
                nc.sync.dma_start(xt_sb[:, k, :], xt_r[:, k, :])
            whh_r = whh.rearrange("(k p) m -> p k m", p=P)
            whh_sb = p12.tile([P, HKT, H], f16)
            for k in range(HKT):
                nc.sync.dma_start(whh_sb[:, k, :], whh_r[:, k, :])
            h0_sb = p12.tile([P, HKT, BL], f16)
            nc.sync.dma_start(h0_sb[:], h0.rearrange("(k p) b -> p k b", p=P))
            C_sb = p12.tile([P, HKT, M], f16)

            # ---------------- phase C: input projection ----------------
            for m in range(HKT if "c" in phases else 0):
                pc = psA.tile([P, M], f32, tag="pc")
                for k in range(EKT):
                    nc.tensor.matmul(pc[:], wih_sb[:, k, ts(m, P)],
                                     xt_sb[:, k, :],
                                     start=(k == 0), stop=(k == EKT - 1))
                nc.vector.tensor_copy(C_sb[:, m, :], pc[:])

            # ---------------- phase 1: recurrence ----------------
            for t in range(S if "1" in phases else 0):
                ph = psA.tile([P, HKT, BL], f32, tag="ph")
                for m in range(HKT):
                    for k in range(HKT):
                        rhs = (h0_sb[:, k, :] if t == 0
                               else H_sb[:, k, ts(t - 1, BL)])
                        nc.tensor.matmul(ph[:, m, :],
                                         whh_sb[:, k, ts(m, P)], rhs,
                                         start=(k == 0), stop=False)
                    nc.tensor.matmul(ph[:, m, :], ident_sb[:],
                                     C_sb[:, m, ts(t, BL)],
                                     start=False, stop=True)
                nc.scalar.activation(H_sb[:, :, ts(t, BL)], ph[:], Act.Tanh)

        # ---------------- phase 2: o^T into XT ----------------
        with tc.tile_pool(name="ph2", bufs=1) as p2, \
             tc.tile_pool(name="psB", bufs=1, space="PSUM") as psB:
            who_sb = p2.tile([P, HKT, V], f16)
            nc.sync.dma_start(who_sb[:], who.rearrange("(k p) v -> p k v", p=P))

            for v in range(VT_FULL + 1 if "2" in phases else 0):
                w = P if v < VT_FULL else V_REM
                po = psB.tile([P, M], f32, tag="po", bufs=3)
                for k in range(HKT):
                    nc.tensor.matmul(po[:w, :],
                                     who_sb[:, k, bass.ds(v * P, w)],
                                     H_sb[:, k, :],
                                     start=(k == 0), stop=(k == HKT - 1))
                nc.vector.tensor_copy(XT[:w, v, :], po[:w, :])

            # ---------------- attention ----------------
            ent_sb = p2.tile([P, BL, HKT, S], f16)
            nc.sync.dma_start(ent_sb[:],
                              ent.rearrange("b (k p) s -> p b k s", p=P))
            enn_sb = p2.tile([S, BL, H], f32)
            nc.sync.dma_start(enn_sb[:], enn.rearrange("b s h -> s b h"))

            for b in range(BL if "a" in phases else 0):
                hloc = [H_sb[:, kk, :].rearrange("p (t b) -> p t b", b=BL)[:, :, b]
                        for kk in range(HKT)]
                pscr = psB.tile([S, S], f32, tag="pscr", bufs=2)
                for k in range(HKT):
                    nc.tensor.matmul(pscr[:], ent_sb[:, b, k, :], hloc[k],
                                     start=(k == 0), stop=(k == HKT - 1))
                sc_sb = p2.tile([S, S], f32, tag="scs")
                nc.vector.tensor_scalar_min(sc_sb[:], pscr[:], SCORE_CLAMP)
                ex_sb = p2.tile([S, S], f32, tag="exs")
                nc.scalar.activation(ex_sb[:], sc_sb[:], Act.Exp)
                # column sums broadcast to all partitions via ones-matmul
                psum_bc = psB.tile([P, S], f32, tag="pbc", bufs=1)
                nc.tensor.matmul(psum_bc[:], ones_sb[:], ex_sb[:],
                                 start=True, stop=True)
                rbc_sb = p2.tile([P, S], f32, tag="rbc")
                nc.vector.reciprocal(rbc_sb[:], psum_bc[:])
                for j in range(HKT):
                    # ctx rows 8000+128j..8127+128j straddle the XT tile grid
                    # (8000 % 128 == 64): two M=64 matmuls land each half in
                    # PSUM at the partition offset its XT tile needs.
                    pctx = psB.tile([P, S], f32, tag="pctx", bufs=1)
                    nc.tensor.matmul(pctx[V_REM:, :],
                                     enn_sb[:, b, bass.ds(j * P, P - V_REM)],
                                     ex_sb[:], start=True, stop=True)
                    lo = XT[V_REM:, VT_FULL + j, :].rearrange(
                        "p (t b) -> p t b", b=BL)[:, :, b]
                    nc.vector.tensor_mul(lo, pctx[V_REM:, :], rbc_sb[V_REM:, :])
                    pctx2 = psB.tile([P, S], f32, tag="pctx2", bufs=1)
                    nc.tensor.matmul(pctx2[:V_REM, :],
                                     enn_sb[:, b, bass.ds(j * P + P - V_REM, V_REM)],
                                     ex_sb[:], start=True, stop=True)
                    hi = XT[:V_REM, VT_FULL + j + 1, :].rearrange(
                        "p (t b) -> p t b", b=BL)[:, :, b]
                    nc.vector.tensor_mul(hi, pctx2[:V_REM, :], rbc_sb[:V_REM, :])

        if debug:
            nc.sync.dma_start(hdump[:], H_sb[:])
            nc.sync.dma_start(xdump[:], XT[:])

        # ---------------- phase 3: big W_c matmul ----------------
        with tc.tile_pool(name="ph3", bufs=6) as p3, \
             tc.tile_pool(name="ph3o", bufs=4) as p3o, \
             tc.tile_pool(name="psC", bufs=8, space="PSUM") as psC:
            for n in range(NCH if "3" in phases else 0):
                pts = [psC.tile([P, NW], f32, tag="p3", name=f"p3_{n}_{m}")
                       for m in range(HKT)]
                for k in range(K3T):
                    rhs = p3.tile([P, NW], f16, tag="rhs")
                    nc.sync.dma_start(rhs[:], wct[k, n])
                    for m in range(HKT):
                        nc.tensor.matmul(pts[m][:], XT[:, k, ts(m, P)], rhs[:],
                                         start=(k == 0), stop=(k == K3T - 1))
                for m in range(HKT):
                    ot = p3o.tile([P, NW], f32, tag="ot")
                    nc.scalar.activation(ot[:], pts[m][:], Act.Tanh)
                    nc.sync.dma_start(out[ts(m, P), ts(n, NW)], ot[:])

    nc.compile()
    return nc


def _host_prep(target, encoder_hiddens, emb, W_ih, W_hh, b_cell, W_ho, b_ho,
               W_c, b_c):
    f16 = np.float16
    tok = np.asarray(target).astype(np.int64).copy()
    tok[:, 0] = SOS_IDX
    x = np.asarray(emb, np.float32)[tok]            # (B, S, E)
    enc = np.asarray(encoder_hiddens, np.float32)

    wih_a = np.zeros((EKT * P, H), f16)
    wih_a[:E] = W_ih.T.astype(f16)                  # (E, H)
    wih_a[E] = np.asarray(b_cell, np.float32).astype(f16)
    whh_a = np.asarray(W_hh, np.float32).T.astype(f16)
    who_a = np.ascontiguousarray(np.asarray(W_ho, np.float32).T).astype(f16)

    W_c = np.asarray(W_c, np.float32)
    b_c_eff = np.asarray(b_c, np.float32) + W_c[:, :V] @ np.asarray(b_ho, np.float32)
    wct_a = np.zeros((K3, V), f16)
    wct_a[:V] = W_c[:, :V].T.astype(f16)
    wct_a[V:V + H] = W_c[:, V:].T.astype(f16)
    wct_a[ONES_ROW] = b_c_eff.astype(f16)
    # blocked layout: (k-tile, n-chunk, partition, col), each DMA tile contiguous
    wct_a = np.ascontiguousarray(
        wct_a.reshape(K3T, P, NCH, NW).transpose(0, 2, 1, 3))

    ident_a = np.eye(P, dtype=f16)

    shared = {"wih": wih_a, "whh": whh_a, "who": who_a, "wct": wct_a,
              "ident": ident_a}

    in_maps = []
    for c in range(NCORES):
        sl = slice(c * BL, (c + 1) * BL)
        xb = x[sl]                                   # (BL, S, E)
        xt_a = np.zeros((EKT * P, M), f16)
        # col r = t*BL + b
        xt_a[:E] = xb.transpose(2, 1, 0).reshape(E, M).astype(f16)
        xt_a[E] = 1.0
        encb = enc[sl]                               # (BL, S, H)
        in_maps.append(dict(
            shared,
            xt=xt_a,
            h0=np.ascontiguousarray(encb[:, -1].T).astype(f16),
            ent=np.ascontiguousarray(encb.transpose(0, 2, 1)).astype(f16),
            enn=np.ascontiguousarray(encb),
        ))
    return in_maps


def _get_program():
    if "nc" not in _cache:
        _cache["nc"] = _build_program(os.environ.get("KERNEL_PHASES", "c12a3"))
    return _cache["nc"]


def kernel(**inputs):
    from concourse.bass_utils import run_bass_kernel_spmd

    nc = _get_program()
    in_maps = _host_prep(**inputs)
    res = run_bass_kernel_spmd(nc, in_maps, core_ids=list(range(NCORES)))
    _cache["last_result"] = res

    outp = np.empty((B, S, V), np.float32)
    for c in range(NCORES):
        loc = res.results[c]["out"]                  # (M, V), r = t*BL + b
        outp[c * BL:(c + 1) * BL] = loc.reshape(S, BL, V).transpose(1, 0, 2)
    return outp


if __name__ == "__main__":
    rng = np.random.default_rng(0)
    ins = {
        "target": rng.integers(0, V, (B, S)),
        "encoder_hiddens": rng.standard_normal((B, S, H)).astype(np.float32),
        "emb": rng.standard_normal((V, E)).astype(np.float32),
        "W_ih": (rng.standard_normal((H, E)) / np.sqrt(E)).astype(np.float32),
        "W_hh": (rng.standard_normal((H, H)) / np.sqrt(H)).astype(np.float32),
        "b_cell": np.zeros(H, np.float32),
        "W_ho": (rng.standard_normal((V, H)) / np.sqrt(H)).astype(np.float32),
        "b_ho": np.zeros(V, np.float32),
        "W_c": (rng.standard_normal((V, V + H)) / np.sqrt(V + H)).astype(np.float32),
        "b_c": np.zeros(V, np.float32),
    }
    o = kernel(**ins)
    print("kernel ran, output shape", o.shape, "finite:", np.isfinite(o).all())


# revision 14
# speedup vs baseline: 1.0973x; 1.0430x over previous
"""Trainium2 Bass kernel for nn_Decoder (RNN decoder w/ Luong attention).

Reference computation (B=64, S=64, H=E=512, V=8000):
    tokens  = [SOS, target[:, 1:]]
    x_seq   = emb[tokens]
    h_0     = encoder_hiddens[:, -1]
    per step t:
        h_t    = tanh(x_t @ W_ih.T + h_{t-1} @ W_hh.T + b_cell)
        o_t    = h_t @ W_ho.T + b_ho
        scores = einsum("bsh,bh->bs", enc, h_t); w = softmax(scores)
        ctx_t  = einsum("bs,bsh->bh", w, enc)
        out_t  = tanh([o_t, ctx_t] @ W_c.T + b_c)

Strategy: fully data-parallel over batch across 8 cores (8 batch rows each),
weights replicated, zero collectives. Per core, with M = 8*64 = 512 local
(t, b) rows:
  phase C: C^T = W_ihT_aug.T @ XT_aug  (input projection for all steps,
           b_cell folded in via an augmented ones row)
  phase 1: sequential recurrence, feature-major h^T (H on partitions,
           batch on free); C added in PSUM via an identity matmul; one
           tanh ACT per step.
  phase 2: o^T = W_ho^T.T @ H  written directly into the phase-3
           stationary buffer XT (no transposes anywhere).
  attn:    scores computed transposed (s on partitions) so the softmax
           denominator comes from a ones-matmul; ctx^T lands in XT.
  phase 3: OUT = XT.T @ W_cT_pad streamed from HBM once (fp16, ~139MB),
           b_c (+ W_c[:, :V] @ b_ho, folded on host) via the ones row,
           tanh on ACT, fp32 out.

All matmul operands fp16 (full PE rate, 10-bit mantissa); softmax exp/ctx
path in fp32.
"""

import os
from contextlib import ExitStack

import numpy as np

# ---- problem constants (hardcoded per harness contract) ----
B, S, H, E, V = 64, 64, 512, 512, 8000
SOS_IDX = 1
NCORES = 8
BL = B // NCORES          # local batch = 8
M = BL * S                # local rows = 512, r = t*BL + b
P = 128                   # partitions

EKT = 5                   # k-tiles for E+bias (640 rows)
HKT = H // P              # 4
# phase-3 contraction layout: [o (8000) | ctx (512) | ones row | zero pad]
K3 = 8576                 # total phase-3 contraction rows (67 tiles)
K3T = K3 // P             # 67
ONES_ROW = V + H          # 8512 (tile 66, partition 64)
VT_FULL = V // P          # 62 full 128-row vocab tiles for o^T
V_REM = V - VT_FULL * P   # 64
NW = 500                  # phase-3 vocab chunk width
NCH = V // NW             # 16
SCORE_CLAMP = 80.0

_f16 = None
_f32 = None

_cache = {}


def _build_program(phases="c123"):
    import concourse.bass as bass
    import concourse.tile as tile
    import concourse.mybir as mybir
    from concourse import bacc

    f16 = mybir.dt.float16
    f32 = mybir.dt.float32
    Act = mybir.ActivationFunctionType

    nc = bacc.Bacc("TRN2", target_bir_lowering=False, debug=False,
                   num_devices=NCORES)

    xt = nc.dram_tensor("xt", [EKT * P, M], f16, kind="ExternalInput").ap()
    wih = nc.dram_tensor("wih", [EKT * P, H], f16, kind="ExternalInput").ap()
    whh = nc.dram_tensor("whh", [H, H], f16, kind="ExternalInput").ap()
    h0 = nc.dram_tensor("h0", [H, BL], f16, kind="ExternalInput").ap()
    ent = nc.dram_tensor("ent", [BL, H, S], f16, kind="ExternalInput").ap()
    enn = nc.dram_tensor("enn", [BL, S, H], f16, kind="ExternalInput").ap()
    who = nc.dram_tensor("who", [H, V], f16, kind="ExternalInput").ap()
    wct = nc.dram_tensor("wct", [K3T, NCH, P, NW], f16,
                         kind="ExternalInput").ap()
    ident = nc.dram_tensor("ident", [P, P], f16, kind="ExternalInput").ap()
    out = nc.dram_tensor("out", [M, V], f32, kind="ExternalOutput").ap()
    debug = os.environ.get("KERNEL_DEBUG_OUT", "0") == "1"
    if debug:
        hdump = nc.dram_tensor("hdump", [P, HKT, M], f16,
                               kind="ExternalOutput").ap()
        xdump = nc.dram_tensor("xdump", [P, K3T, M], f16,
                               kind="ExternalOutput").ap()

    with tile.TileContext(nc) as tc, ExitStack() as ctx:
        ts = bass.ts

        # ---------------- persistent SBUF ----------------
        const_pool = ctx.enter_context(tc.tile_pool(name="consts", bufs=1))
        xt_pool = ctx.enter_context(tc.tile_pool(name="xtp", bufs=1))

        ident_sb = const_pool.tile([P, P], f16)
        nc.sync.dma_start(ident_sb[:], ident[:])
        ones_sb = const_pool.tile([S, P], f32)   # softmax-sum broadcaster
        nc.vector.memset(ones_sb[:], 1.0)

        XT = xt_pool.tile([P, K3T, M], f16)      # phase-3 stationary
        H_sb = xt_pool.tile([P, HKT, M], f16)    # h^T for all local rows
        who_sb = xt_pool.tile([P, HKT, V], f16)

        # zero XT pad region + ones row (rest fully written by phases 2/attn)
        nc.vector.memset(XT[V_REM:, K3T - 1, :], 0.0)
        nc.vector.memset(XT[V_REM:V_REM + 1, K3T - 1, :], 1.0)

        with tc.tile_pool(name="ph12", bufs=1) as p12, \
             tc.tile_pool(name="psA", bufs=2, space="PSUM") as psA:
            xt_r = xt.rearrange("(k p) m -> p k m", p=P)
            xt_sb = p12.tile([P, EKT, M], f16)
            wih_r = wih.rearrange("(k p) m -> p k m", p=P)
            wih_sb = p12.tile([P, EKT, H], f16)
            for k in range(EKT):
                nc.sync.dma_start(wih_sb[:, k, :], wih_r[:, k, :])
                nc.sync.dma_start(xt_sb[:, k, :], xt_r[:, k, :])
            whh_r = whh.rearrange("(k p) m -> p k m", p=P)
            whh_sb = p12.tile([P, HKT, H], f16)
            for k in range(HKT):
                nc.sync.dma_start(whh_sb[:, k, :], whh_r[:, k, :])
            h0_sb = p12.tile([P, HKT, BL], f16)
            nc.sync.dma_start(h0_sb[:], h0.rearrange("(k p) b -> p k b", p=P))
            who_r = who.rearrange("(k p) v -> p k v", p=P)
            for k in range(HKT):
                nc.sync.dma_start(who_sb[:, k, :], who_r[:, k, :])
            C_sb = p12.tile([P, HKT, M], f16)

            # ---------------- phase C: input projection ----------------
            for m in range(HKT if "c" in phases else 0):
                pc = psA.tile([P, M], f32, tag="pc")
                for k in range(EKT):
                    nc.tensor.matmul(pc[:], wih_sb[:, k, ts(m, P)],
                                     xt_sb[:, k, :],
                                     start=(k == 0), stop=(k == EKT - 1))
                nc.vector.tensor_copy(C_sb[:, m, :], pc[:])

            # ---------------- phase 1: recurrence ----------------
            for t in range(S if "1" in phases else 0):
                ph = psA.tile([P, HKT, BL], f32, tag="ph")
                for m in range(HKT):
                    for k in range(HKT):
                        rhs = (h0_sb[:, k, :] if t == 0
                               else H_sb[:, k, ts(t - 1, BL)])
                        nc.tensor.matmul(ph[:, m, :],
                                         whh_sb[:, k, ts(m, P)], rhs,
                                         start=(k == 0), stop=False)
                    nc.tensor.matmul(ph[:, m, :], ident_sb[:],
                                     C_sb[:, m, ts(t, BL)],
                                     start=False, stop=True)
                nc.scalar.activation(H_sb[:, :, ts(t, BL)], ph[:], Act.Tanh)

        # ---------------- phase 2: o^T into XT ----------------
        with tc.tile_pool(name="ph2", bufs=1) as p2, \
             tc.tile_pool(name="psB", bufs=1, space="PSUM") as psB:
            def o_pass(c0, c1, name):
                for v in range(VT_FULL + 1):
                    w = P if v < VT_FULL else V_REM
                    po = psB.tile([P, c1 - c0], f32, tag="po", bufs=3,
                                  name=f"po_{name}_{v}")
                    for k in range(HKT):
                        nc.tensor.matmul(po[:w, :],
                                         who_sb[:, k, bass.ds(v * P, w)],
                                         H_sb[:, k, c0:c1],
                                         start=(k == 0), stop=(k == HKT - 1))
                    nc.vector.tensor_copy(XT[:w, v, c0:c1], po[:w, :])

            if "2" in phases:
                o_pass(0, M // 2, "a")   # t < 32: schedulable into recurrence tail

            # ---------------- attention ----------------
            ent_sb = p2.tile([P, BL, HKT, S], f16)
            nc.sync.dma_start(ent_sb[:],
                              ent.rearrange("b (k p) s -> p b k s", p=P))
            enn_sb = p2.tile([S, BL, H], f16)
            nc.sync.dma_start(enn_sb[:], enn.rearrange("b s h -> s b h"))

            for b in range(BL if "a" in phases else 0):
                hloc = [H_sb[:, kk, :].rearrange("p (t b) -> p t b", b=BL)[:, :, b]
                        for kk in range(HKT)]
                pscr = psB.tile([S, S], f32, tag="pscr", bufs=2)
                for k in range(HKT):
                    nc.tensor.matmul(pscr[:], ent_sb[:, b, k, :], hloc[k],
                                     start=(k == 0), stop=(k == HKT - 1))
                sc_sb = p2.tile([S, S], f32, tag="scs", bufs=2)
                nc.vector.tensor_scalar_min(sc_sb[:], pscr[:], SCORE_CLAMP)
                ex_sb = p2.tile([S, S], f32, tag="exs", bufs=2)
                nc.scalar.activation(ex_sb[:], sc_sb[:], Act.Exp)
                # column sums broadcast to all partitions via ones-matmul
                psum_bc = psB.tile([P, S], f32, tag="pctx", bufs=3,
                                   name=f"pbc_{b}")
                nc.tensor.matmul(psum_bc[:S, :], ones_sb[:, :S], ex_sb[:],
                                 start=True, stop=True)
                rbc_sb = p2.tile([S, S], f32, tag="rbc", bufs=2)
                nc.vector.reciprocal(rbc_sb[:], psum_bc[:S, :])
                # normalized weights in fp16 -> whole ctx path runs fp16
                exn_sb = p2.tile([S, S], f16, tag="exn", bufs=2)
                nc.vector.tensor_mul(exn_sb[:], ex_sb[:], rbc_sb[:])
                for j in range(HKT):
                    # ctx rows 8000+128j..8127+128j straddle the XT tile grid
                    # (8000 % 128 == 64): two M=64 matmuls land each half in
                    # PSUM at the partition offset its XT tile needs.
                    pctx = psB.tile([P, S], f32, tag="pctx", bufs=3,
                                    name=f"pctx_{b}_{j}")
                    nc.tensor.matmul(pctx[V_REM:, :],
                                     enn_sb[:, b, bass.ds(j * P, P - V_REM)],
                                     exn_sb[:], start=True, stop=True)
                    lo = XT[V_REM:, VT_FULL + j, :].rearrange(
                        "p (t b) -> p t b", b=BL)[:, :, b]
                    nc.vector.tensor_copy(lo, pctx[V_REM:, :])
                    nc.tensor.matmul(pctx[:V_REM, :],
                                     enn_sb[:, b, bass.ds(j * P + P - V_REM, V_REM)],
                                     exn_sb[:], start=True, stop=True)
                    hi = XT[:V_REM, VT_FULL + j + 1, :].rearrange(
                        "p (t b) -> p t b", b=BL)[:, :, b]
                    nc.vector.tensor_copy(hi, pctx[:V_REM, :])

            if "2" in phases:
                o_pass(M // 2, M, "b")   # t >= 32: gap-fills attention stalls

        if debug:
            nc.sync.dma_start(hdump[:], H_sb[:])
            nc.sync.dma_start(xdump[:], XT[:])

        # ---------------- phase 3: big W_c matmul ----------------
        with tc.tile_pool(name="ph3", bufs=6) as p3, \
             tc.tile_pool(name="ph3o", bufs=4) as p3o, \
             tc.tile_pool(name="psC", bufs=8, space="PSUM") as psC:
            for n in range(NCH if "3" in phases else 0):
                pts = [psC.tile([P, NW], f32, tag="p3", name=f"p3_{n}_{m}")
                       for m in range(HKT)]
                for k in range(K3T):
                    rhs = p3.tile([P, NW], f16, tag="rhs")
                    nc.sync.dma_start(rhs[:], wct[k, n])
                    for m in range(HKT):
                        nc.tensor.matmul(pts[m][:], XT[:, k, ts(m, P)], rhs[:],
                                         start=(k == 0), stop=(k == K3T - 1))
                for m in range(HKT):
                    ot = p3o.tile([P, NW], f32, tag="ot")
                    nc.scalar.activation(ot[:], pts[m][:], Act.Tanh)
                    nc.sync.dma_start(out[ts(m, P), ts(n, NW)], ot[:])

    nc.compile()
    return nc


def _host_prep(target, encoder_hiddens, emb, W_ih, W_hh, b_cell, W_ho, b_ho,
               W_c, b_c):
    f16 = np.float16
    tok = np.asarray(target).astype(np.int64).copy()
    tok[:, 0] = SOS_IDX
    x = np.asarray(emb, np.float32)[tok]            # (B, S, E)
    enc = np.asarray(encoder_hiddens, np.float32)

    wih_a = np.zeros((EKT * P, H), f16)
    wih_a[:E] = W_ih.T.astype(f16)                  # (E, H)
    wih_a[E] = np.asarray(b_cell, np.float32).astype(f16)
    whh_a = np.asarray(W_hh, np.float32).T.astype(f16)
    who_a = np.ascontiguousarray(np.asarray(W_ho, np.float32).T).astype(f16)

    W_c = np.asarray(W_c, np.float32)
    b_c_eff = np.asarray(b_c, np.float32) + W_c[:, :V] @ np.asarray(b_ho, np.float32)
    wct_a = np.zeros((K3, V), f16)
    wct_a[:V] = W_c[:, :V].T.astype(f16)
    wct_a[V:V + H] = W_c[:, V:].T.astype(f16)
    wct_a[ONES_ROW] = b_c_eff.astype(f16)
    # blocked layout: (k-tile, n-chunk, partition, col), each DMA tile contiguous
    wct_a = np.ascontiguousarray(
        wct_a.reshape(K3T, P, NCH, NW).transpose(0, 2, 1, 3))

    ident_a = np.eye(P, dtype=f16)

    shared = {"wih": wih_a, "whh": whh_a, "who": who_a, "wct": wct_a,
              "ident": ident_a}

    in_maps = []
    for c in range(NCORES):
        sl = slice(c * BL, (c + 1) * BL)
        xb = x[sl]                                   # (BL, S, E)
        xt_a = np.zeros((EKT * P, M), f16)
        # col r = t*BL + b
        xt_a[:E] = xb.transpose(2, 1, 0).reshape(E, M).astype(f16)
        xt_a[E] = 1.0
        encb = enc[sl]                               # (BL, S, H)
        in_maps.append(dict(
            shared,
            xt=xt_a,
            h0=np.ascontiguousarray(encb[:, -1].T).astype(f16),
            ent=np.ascontiguousarray(encb.transpose(0, 2, 1)).astype(f16),
            enn=np.ascontiguousarray(encb).astype(f16),
        ))
    return in_maps


def _get_program():
    if "nc" not in _cache:
        _cache["nc"] = _build_program(os.environ.get("KERNEL_PHASES", "c12a3"))
    return _cache["nc"]


def kernel(**inputs):
    from concourse.bass_utils import run_bass_kernel_spmd

    nc = _get_program()
    in_maps = _host_prep(**inputs)
    res = run_bass_kernel_spmd(nc, in_maps, core_ids=list(range(NCORES)))
    _cache["last_result"] = res

    outp = np.empty((B, S, V), np.float32)
    for c in range(NCORES):
        loc = res.results[c]["out"]                  # (M, V), r = t*BL + b
        outp[c * BL:(c + 1) * BL] = loc.reshape(S, BL, V).transpose(1, 0, 2)
    return outp


if __name__ == "__main__":
    rng = np.random.default_rng(0)
    ins = {
        "target": rng.integers(0, V, (B, S)),
        "encoder_hiddens": rng.standard_normal((B, S, H)).astype(np.float32),
        "emb": rng.standard_normal((V, E)).astype(np.float32),
        "W_ih": (rng.standard_normal((H, E)) / np.sqrt(E)).astype(np.float32),
        "W_hh": (rng.standard_normal((H, H)) / np.sqrt(H)).astype(np.float32),
        "b_cell": np.zeros(H, np.float32),
        "W_ho": (rng.standard_normal((V, H)) / np.sqrt(H)).astype(np.float32),
        "b_ho": np.zeros(V, np.float32),
        "W_c": (rng.standard_normal((V, V + H)) / np.sqrt(V + H)).astype(np.float32),
        "b_c": np.zeros(V, np.float32),
    }
    o = kernel(**ins)
    print("kernel ran, output shape", o.shape, "finite:", np.isfinite(o).all())


# revision 20
# speedup vs baseline: 1.1253x; 1.0255x over previous
"""Trainium2 Bass kernel for nn_Decoder (RNN decoder w/ Luong attention).

Reference computation (B=64, S=64, H=E=512, V=8000):
    tokens  = [SOS, target[:, 1:]]
    x_seq   = emb[tokens]
    h_0     = encoder_hiddens[:, -1]
    per step t:
        h_t    = tanh(x_t @ W_ih.T + h_{t-1} @ W_hh.T + b_cell)
        o_t    = h_t @ W_ho.T + b_ho
        scores = einsum("bsh,bh->bs", enc, h_t); w = softmax(scores)
        ctx_t  = einsum("bs,bsh->bh", w, enc)
        out_t  = tanh([o_t, ctx_t] @ W_c.T + b_c)

Strategy: fully data-parallel over batch across 8 cores (8 batch rows each),
weights replicated, zero collectives. Per core, with M = 8*64 = 512 local
(t, b) rows:
  phase C: C^T = W_ihT_aug.T @ XT_aug  (input projection for all steps,
           b_cell folded in via an augmented ones row)
  phase 1: sequential recurrence, feature-major h^T (H on partitions,
           batch on free); C added in PSUM via an identity matmul; one
           tanh ACT per step.
  phase 2: o^T = W_ho^T.T @ H  written directly into the phase-3
           stationary buffer XT (no transposes anywhere).
  attn:    scores computed transposed (s on partitions) so the softmax
           denominator comes from a ones-matmul; ctx^T lands in XT.
  phase 3: OUT = XT.T @ W_cT_pad streamed from HBM once (fp16, ~139MB),
           b_c (+ W_c[:, :V] @ b_ho, folded on host) via the ones row,
           tanh on ACT, fp32 out.

All matmul operands fp16 (full PE rate, 10-bit mantissa); softmax exp/ctx
path in fp32.
"""

import os
from contextlib import ExitStack

import numpy as np

# ---- problem constants (hardcoded per harness contract) ----
B, S, H, E, V = 64, 64, 512, 512, 8000
SOS_IDX = 1
NCORES = 8
BL = B // NCORES          # local batch = 8
M = BL * S                # local rows = 512, r = t*BL + b
P = 128                   # partitions

EKT = 5                   # k-tiles for E+bias (640 rows)
HKT = H // P              # 4
# phase-3 contraction layout: [o (8000) | ctx (512) | ones row | zero pad]
K3 = 8576                 # total phase-3 contraction rows (67 tiles)
K3T = K3 // P             # 67
ONES_ROW = V + H          # 8512 (tile 66, partition 64)
VT_FULL = V // P          # 62 full 128-row vocab tiles for o^T
V_REM = V - VT_FULL * P   # 64
NW = 500                  # phase-3 vocab chunk width
NCH = V // NW             # 16
SCORE_CLAMP = 80.0

_f16 = None
_f32 = None

_cache = {}


def _build_program(phases="c123"):
    import concourse.bass as bass
    import concourse.tile as tile
    import concourse.mybir as mybir
    from concourse import bacc

    f16 = mybir.dt.float16
    f32 = mybir.dt.float32
    Act = mybir.ActivationFunctionType

    nc = bacc.Bacc("TRN2", target_bir_lowering=False, debug=False,
                   num_devices=NCORES)

    xt = nc.dram_tensor("xt", [EKT * P, M], f16, kind="ExternalInput").ap()
    wih = nc.dram_tensor("wih", [EKT * P, H], f16, kind="ExternalInput").ap()
    whh = nc.dram_tensor("whh", [H, H], f16, kind="ExternalInput").ap()
    h0 = nc.dram_tensor("h0", [H, BL], f16, kind="ExternalInput").ap()
    ent = nc.dram_tensor("ent", [BL, H, S], f16, kind="ExternalInput").ap()
    enn = nc.dram_tensor("enn", [BL, S, H], f16, kind="ExternalInput").ap()
    who = nc.dram_tensor("who", [H, V], f16, kind="ExternalInput").ap()
    wct = nc.dram_tensor("wct", [K3T, NCH, P, NW], f16,
                         kind="ExternalInput").ap()
    ident = nc.dram_tensor("ident", [P, P], f16, kind="ExternalInput").ap()
    out = nc.dram_tensor("out", [M, V], f32, kind="ExternalOutput").ap()
    debug = os.environ.get("KERNEL_DEBUG_OUT", "0") == "1"
    if debug:
        hdump = nc.dram_tensor("hdump", [P, S, HKT, BL], f16,
                               kind="ExternalOutput").ap()
        xdump = nc.dram_tensor("xdump", [P, K3T, M], f16,
                               kind="ExternalOutput").ap()

    with tile.TileContext(nc) as tc, ExitStack() as ctx:
        ts = bass.ts

        # ---------------- persistent SBUF ----------------
        const_pool = ctx.enter_context(tc.tile_pool(name="consts", bufs=1))
        xt_pool = ctx.enter_context(tc.tile_pool(name="xtp", bufs=1))

        ident_sb = const_pool.tile([P, P], f16)
        nc.sync.dma_start(ident_sb[:], ident[:])
        ones_sb = const_pool.tile([S, P], f32)   # softmax-sum broadcaster
        nc.vector.memset(ones_sb[:], 1.0)

        XT = xt_pool.tile([P, K3T, M], f16)      # phase-3 stationary
        # h^T for all local rows; [p, t, k, b] so each step's tanh write is
        # one contiguous range (keeps region deps precise for the o^T passes)
        H_sb = xt_pool.tile([P, S, HKT, BL], f16)
        who_sb = xt_pool.tile([P, HKT, V], f16)

        # zero XT pad region + ones row (rest fully written by phases 2/attn)
        nc.vector.memset(XT[V_REM:, K3T - 1, :], 0.0)
        nc.vector.memset(XT[V_REM:V_REM + 1, K3T - 1, :], 1.0)

        with tc.tile_pool(name="ph12", bufs=1) as p12, \
             tc.tile_pool(name="psAB", bufs=1, space="PSUM") as psA:
            xt_r = xt.rearrange("(k p) m -> p k m", p=P)
            xt_sb = p12.tile([P, EKT, M], f16)
            wih_r = wih.rearrange("(k p) m -> p k m", p=P)
            wih_sb = p12.tile([P, EKT, H], f16)
            for k in range(EKT):
                nc.sync.dma_start(wih_sb[:, k, :], wih_r[:, k, :])
                nc.sync.dma_start(xt_sb[:, k, :], xt_r[:, k, :])
            whh_r = whh.rearrange("(k p) m -> p k m", p=P)
            whh_sb = p12.tile([P, HKT, H], f16)
            for k in range(HKT):
                nc.sync.dma_start(whh_sb[:, k, :], whh_r[:, k, :])
            h0_sb = p12.tile([P, HKT, BL], f16)
            nc.sync.dma_start(h0_sb[:], h0.rearrange("(k p) b -> p k b", p=P))
            who_r = who.rearrange("(k p) v -> p k v", p=P)
            for k in range(HKT):
                nc.sync.dma_start(who_sb[:, k, :], who_r[:, k, :])
            C_sb = p12.tile([P, HKT, M], f16)

            # ---------------- phase C: input projection ----------------
            for m in range(HKT if "c" in phases else 0):
                pc = psA.tile([P, M], f32, tag="pc",
                              bufs=int(os.environ.get("PO_BUFS", "3")))
                for k in range(EKT):
                    nc.tensor.matmul(pc[:], wih_sb[:, k, ts(m, P)],
                                     xt_sb[:, k, :],
                                     start=(k == 0), stop=(k == EKT - 1))
                nc.vector.tensor_copy(C_sb[:, m, :], pc[:])

            # ---------------- phase 1: recurrence ----------------
            for t in range(S if "1" in phases else 0):
                ph = psA.tile([P, HKT, BL], f32, tag="ph", bufs=2)
                for m in range(HKT):
                    for k in range(HKT):
                        rhs = (h0_sb[:, k, :] if t == 0
                               else H_sb[:, t - 1, k, :])
                        nc.tensor.matmul(ph[:, m, :],
                                         whh_sb[:, k, ts(m, P)], rhs,
                                         start=(k == 0), stop=False)
                    nc.tensor.matmul(ph[:, m, :], ident_sb[:],
                                     C_sb[:, m, ts(t, BL)],
                                     start=False, stop=True)
                nc.scalar.activation(H_sb[:, t, :, :], ph[:], Act.Tanh)

            # ------------- phase 2: o^T into XT (same psum pool) -------------
            def o_pass(c0, c1, name):
                for v in range(VT_FULL + 1):
                    w = P if v < VT_FULL else V_REM
                    po = psA.tile([P, c1 - c0], f32, tag="pc",
                                  bufs=int(os.environ.get("PO_BUFS", "3")),
                                  name=f"po_{name}_{v}")
                    for k in range(HKT):
                        nc.tensor.matmul(po[:w, :],
                                         who_sb[:, k, bass.ds(v * P, w)],
                                         H_sb[:, c0 // BL:c1 // BL, k, :],
                                         start=(k == 0), stop=(k == HKT - 1))
                    nc.vector.tensor_copy(XT[:w, v, c0:c1], po[:w, :])

            if "2" in phases:
                o_pass(0, M // 2, "a")   # t < 32: schedulable into recurrence tail

            # ---------------- attention ----------------
            ent_sb = xt_pool.tile([P, BL, HKT, S], f16)
            nc.sync.dma_start(ent_sb[:],
                              ent.rearrange("b (k p) s -> p b k s", p=P))
            enn_sb = xt_pool.tile([S, BL, H], f16)
            nc.sync.dma_start(enn_sb[:], enn.rearrange("b s h -> s b h"))

            for b in range(BL if "a" in phases else 0):
                hloc = [H_sb[:, :, kk, b] for kk in range(HKT)]
                pscr = psA.tile([S, S], f32, tag="pscr",
                                bufs=int(os.environ.get("PSCR_BUFS", "1")))
                for k in range(HKT):
                    nc.tensor.matmul(pscr[:], ent_sb[:, b, k, :], hloc[k],
                                     start=(k == 0), stop=(k == HKT - 1))
                sc_sb = p12.tile([S, S], f32, tag="scs", bufs=2)
                nc.vector.tensor_scalar_min(sc_sb[:], pscr[:], SCORE_CLAMP)
                ex_sb = p12.tile([S, S], f32, tag="exs", bufs=2)
                nc.scalar.activation(ex_sb[:], sc_sb[:], Act.Exp)
                # column sums broadcast to all partitions via ones-matmul
                psum_bc = psA.tile([P, S], f32, tag="pctx",
                                   bufs=int(os.environ.get("PCTX_BUFS", "2")),
                                   name=f"pbc_{b}")
                nc.tensor.matmul(psum_bc[:S, :], ones_sb[:, :S], ex_sb[:],
                                 start=True, stop=True)
                rbc_sb = p12.tile([S, S], f32, tag="rbc", bufs=2)
                nc.vector.reciprocal(rbc_sb[:], psum_bc[:S, :])
                # normalized weights in fp16 -> whole ctx path runs fp16
                exn_sb = p12.tile([S, S], f16, tag="exn", bufs=2)
                nc.vector.tensor_mul(exn_sb[:], ex_sb[:], rbc_sb[:])
                for j in range(HKT):
                    # ctx rows 8000+128j..8127+128j straddle the XT tile grid
                    # (8000 % 128 == 64): two M=64 matmuls land each half in
                    # PSUM at the partition offset its XT tile needs.
                    pctx = psA.tile([P, S], f32, tag="pctx",
                                    bufs=int(os.environ.get("PCTX_BUFS", "2")),
                                    name=f"pctx_{b}_{j}")
                    nc.tensor.matmul(pctx[V_REM:, :],
                                     enn_sb[:, b, bass.ds(j * P, P - V_REM)],
                                     exn_sb[:], start=True, stop=True)
                    lo = XT[V_REM:, VT_FULL + j, :].rearrange(
                        "p (t b) -> p t b", b=BL)[:, :, b]
                    nc.vector.tensor_copy(lo, pctx[V_REM:, :])
                    nc.tensor.matmul(pctx[:V_REM, :],
                                     enn_sb[:, b, bass.ds(j * P + P - V_REM, V_REM)],
                                     exn_sb[:], start=True, stop=True)
                    hi = XT[:V_REM, VT_FULL + j + 1, :].rearrange(
                        "p (t b) -> p t b", b=BL)[:, :, b]
                    nc.vector.tensor_copy(hi, pctx[:V_REM, :])

            if "2" in phases:
                o_pass(M // 2, M, "b")   # t >= 32: gap-fills attention stalls

        if debug:
            nc.sync.dma_start(hdump[:], H_sb[:])
            nc.sync.dma_start(xdump[:], XT[:])

        # ---------------- phase 3: big W_c matmul ----------------
        with tc.tile_pool(name="ph3", bufs=int(os.environ.get("RHS_BUFS", "10"))) as p3, \
             tc.tile_pool(name="ph3o", bufs=4) as p3o, \
             tc.tile_pool(name="psC", bufs=8, space="PSUM") as psC:
            for n in range(NCH if "3" in phases else 0):
                pts = [psC.tile([P, NW], f32, tag="p3", name=f"p3_{n}_{m}")
                       for m in range(HKT)]
                for k in range(K3T):
                    rhs = p3.tile([P, NW], f16, tag="rhs")
                    nc.sync.dma_start(rhs[:], wct[k, n])
                    for m in range(HKT):
                        nc.tensor.matmul(pts[m][:], XT[:, k, ts(m, P)], rhs[:],
                                         start=(k == 0), stop=(k == K3T - 1))
                for m in range(HKT):
                    ot = p3o.tile([P, NW], f32, tag="ot")
                    nc.scalar.activation(ot[:], pts[m][:], Act.Tanh)
                    nc.sync.dma_start(out[ts(m, P), ts(n, NW)], ot[:])

    nc.compile()
    return nc


def _host_prep(target, encoder_hiddens, emb, W_ih, W_hh, b_cell, W_ho, b_ho,
               W_c, b_c):
    f16 = np.float16
    tok = np.asarray(target).astype(np.int64).copy()
    tok[:, 0] = SOS_IDX
    x = np.asarray(emb, np.float32)[tok]            # (B, S, E)
    enc = np.asarray(encoder_hiddens, np.float32)

    wih_a = np.zeros((EKT * P, H), f16)
    wih_a[:E] = W_ih.T.astype(f16)                  # (E, H)
    wih_a[E] = np.asarray(b_cell, np.float32).astype(f16)
    whh_a = np.asarray(W_hh, np.float32).T.astype(f16)
    who_a = np.ascontiguousarray(np.asarray(W_ho, np.float32).T).astype(f16)

    W_c = np.asarray(W_c, np.float32)
    b_c_eff = np.asarray(b_c, np.float32) + W_c[:, :V] @ np.asarray(b_ho, np.float32)
    wct_a = np.zeros((K3, V), f16)
    wct_a[:V] = W_c[:, :V].T.astype(f16)
    wct_a[V:V + H] = W_c[:, V:].T.astype(f16)
    wct_a[ONES_ROW] = b_c_eff.astype(f16)
    # blocked layout: (k-tile, n-chunk, partition, col), each DMA tile contiguous
    wct_a = np.ascontiguousarray(
        wct_a.reshape(K3T, P, NCH, NW).transpose(0, 2, 1, 3))

    ident_a = np.eye(P, dtype=f16)

    shared = {"wih": wih_a, "whh": whh_a, "who": who_a, "wct": wct_a,
              "ident": ident_a}

    in_maps = []
    for c in range(NCORES):
        sl = slice(c * BL, (c + 1) * BL)
        xb = x[sl]                                   # (BL, S, E)
        xt_a = np.zeros((EKT * P, M), f16)
        # col r = t*BL + b
        xt_a[:E] = xb.transpose(2, 1, 0).reshape(E, M).astype(f16)
        xt_a[E] = 1.0
        encb = enc[sl]                               # (BL, S, H)
        in_maps.append(dict(
            shared,
            xt=xt_a,
            h0=np.ascontiguousarray(encb[:, -1].T).astype(f16),
            ent=np.ascontiguousarray(encb.transpose(0, 2, 1)).astype(f16),
            enn=np.ascontiguousarray(encb).astype(f16),
        ))
    return in_maps


def _get_program():
    if "nc" not in _cache:
        _cache["nc"] = _build_program(os.environ.get("KERNEL_PHASES", "c12a3"))
    return _cache["nc"]


def kernel(**inputs):
    from concourse.bass_utils import run_bass_kernel_spmd

    nc = _get_program()
    in_maps = _host_prep(**inputs)
    res = run_bass_kernel_spmd(nc, in_maps, core_ids=list(range(NCORES)))
    _cache["last_result"] = res

    outp = np.empty((B, S, V), np.float32)
    for c in range(NCORES):
        loc = res.results[c]["out"]                  # (M, V), r = t*BL + b
        outp[c * BL:(c + 1) * BL] = loc.reshape(S, BL, V).transpose(1, 0, 2)
    return outp


if __name__ == "__main__":
    rng = np.random.default_rng(0)
    ins = {
        "target": rng.integers(0, V, (B, S)),
        "encoder_hiddens": rng.standard_normal((B, S, H)).astype(np.float32),
        "emb": rng.standard_normal((V, E)).astype(np.float32),
        "W_ih": (rng.standard_normal((H, E)) / np.sqrt(E)).astype(np.float32),
        "W_hh": (rng.standard_normal((H, H)) / np.sqrt(H)).astype(np.float32),
        "b_cell": np.zeros(H, np.float32),
        "W_ho": (rng.standard_normal((V, H)) / np.sqrt(H)).astype(np.float32),
        "b_ho": np.zeros(V, np.float32),
        "W_c": (rng.standard_normal((V, V + H)) / np.sqrt(V + H)).astype(np.float32),
        "b_c": np.zeros(V, np.float32),
    }
    o = kernel(**ins)
    print("kernel ran, output shape", o.shape, "finite:", np.isfinite(o).all())
